# revision 1
# baseline (speedup 1.0000x reference)
"""Trainium2 Bass kernel for a 2-layer GCN (GCNConv x2 + linear head +
log_softmax) distributed across 8 NeuronCores.

Self-contained: accepts FULL inputs, shards internally, runs one SPMD Bass
program on cores 0-7 via run_bass_kernel_spmd, gathers the full output.
"""
"""GCN (2-layer GCNConv + linear head + log_softmax) on 8 TRN2 NeuronCores.

Strategy:
- Nodes sharded across 8 cores (contiguous ranges, padded to NSH mult of 128).
- Per layer: node-sharded transform t = dinv * (h @ W) written as bf16 table
  rows padded to 128 elems (256B); AllGather -> full table on every core.
- Aggregation: each core owns edges whose dst is in its shard. Edges sorted by
  (src-quadrant, dst-window). Per (quadrant, window-group) one dma_gather
  (int16 idx relative to quadrant base) fetches message rows; a 0/1 selection
  matrix S (built on DVE via is_equal against iota) scatter-adds each chunk of
  128 edges into the dst-window PSUM via TensorE matmul: psum += S^T @ msgs.
  norm = dinv[src]*dinv[dst] is factored: dinv[src] into table rows,
  dinv[dst] applied at PSUM evacuation (+ bias + relu fused).
"""
import numpy as np
import ml_dtypes

import concourse.bass as bass
import concourse.tile as tile
from concourse import bacc, mybir

P = 128
NCORES = 8
NQ = 4  # src quadrants (int16 idx limit: rows per quadrant < 32768)

BF16 = ml_dtypes.bfloat16


def ru(x, m):
    return (x + m - 1) // m * m


def preprocess(x, edge_index, W1, b1, W2, b2, W3, b3):
    """Host-side sharding/preprocessing. Returns (meta, in_maps)."""
    x = np.asarray(x, dtype=np.float32)
    ei = np.asarray(edge_index)
    N, IN_DIM = x.shape
    HID = np.asarray(W1).shape[1]
    OUT = np.asarray(W3).shape[1]
    assert N % NCORES == 0
    SH = N // NCORES            # real nodes per shard
    NSH = ru(SH, P)             # padded shard
    NPAD = NCORES * NSH
    assert NPAD % NQ == 0
    QR = NPAD // NQ             # rows per quadrant table
    assert QR < 32768
    NW = NSH // P               # windows per core
    SWG = []                    # window groups (start, count); one PSUM bank
    w0 = 0                      # per window, 6 banks for aggregation
    while w0 < NW:
        nw = min(6, NW - w0)
        SWG.append((w0, nw))
        w0 += nw

    src = np.asarray(ei[0], dtype=np.int64)
    dst = np.asarray(ei[1], dtype=np.int64)
    # self loops
    loop = np.arange(N, dtype=np.int64)
    src = np.concatenate([src, loop])
    dst = np.concatenate([dst, loop])

    deg = np.bincount(dst, minlength=N).astype(np.float32)
    dinv = np.where(deg > 0, 1.0 / np.sqrt(deg), 0.0).astype(np.float32)

    csrc = src // SH
    sgp = csrc * NSH + (src - csrc * SH)      # padded global src id
    cdst = dst // SH
    dl = dst - cdst * SH                      # local dst id (0..SH-1)

    q = sgp // QR                             # src quadrant
    w = dl // P                               # dst window
    slot = dl % P                             # dst slot in window
    rel = (sgp - q * QR).astype(np.int16)     # idx within quadrant table

    # per (core, quadrant, window) counts
    key = (cdst * NQ + q) * NW + w
    counts = np.bincount(key, minlength=NCORES * NQ * NW).reshape(NCORES, NQ, NW)
    caps = ru(counts.max(axis=0), P)          # [NQ, NW]
    TOT = int(caps.sum())
    TOTC = TOT // P

    # static granule table: per (q, window-group)
    granules = []
    off = 0
    for qq in range(NQ):
        for (gw0, gnw) in SWG:
            wcaps = [int(caps[qq, gw0 + i]) for i in range(gnw)]
            n_idx = int(sum(wcaps))
            granules.append(dict(q=qq, w0=gw0, nw=gnw, off=off, n_idx=n_idx,
                                 wchunks=[c // P for c in wcaps]))
            off += n_idx
    assert off == TOT

    # slot offsets per (q,w) in the concatenated stream
    qw_off = np.zeros((NQ, NW), dtype=np.int64)
    o = 0
    for qq in range(NQ):
        for ww in range(NW):
            qw_off[qq, ww] = o
            o += caps[qq, ww]

    order = np.lexsort((w, q, cdst))
    sgp_s, q_s, w_s, slot_s, rel_s, cdst_s = (
        sgp[order], q[order], w[order], slot[order], rel[order], cdst[order])

    meta = dict(N=N, IN_DIM=IN_DIM, HID=HID, OUT=OUT, SH=SH, NSH=NSH, NPAD=NPAD,
                QR=QR, NW=NW, SWG=SWG, granules=granules, TOT=TOT, TOTC=TOTC)

    iota_np = np.broadcast_to(np.arange(P, dtype=np.float32)[None, :], (P, P))
    iota_bf = np.ascontiguousarray(iota_np.astype(BF16))
    ident = np.eye(P, dtype=np.float32)
    W1f = np.asarray(W1, np.float32)
    W2f = np.asarray(W2, np.float32)
    W3f = np.asarray(W3, np.float32)
    b1r = np.broadcast_to(np.asarray(b1, np.float32)[None, :], (P, HID)).copy()
    b2r = np.broadcast_to(np.asarray(b2, np.float32)[None, :], (P, HID)).copy()
    b3r = np.broadcast_to(np.asarray(b3, np.float32)[None, :], (P, OUT)).copy()

    in_maps = []
    for c in range(NCORES):
        m = cdst_s == c
        e_rel = rel_s[m]
        e_slot = slot_s[m]
        e_q = q_s[m]
        e_w = w_s[m]
        # rank of each edge inside its (q,w) segment (edges sorted by (q,w))
        seg_key = e_q * NW + e_w
        ne = len(seg_key)
        rank = np.zeros(ne, dtype=np.int64)
        if ne:
            seg_change = np.empty(ne, dtype=bool)
            seg_change[0] = True
            seg_change[1:] = seg_key[1:] != seg_key[:-1]
            seg_start = np.flatnonzero(seg_change)
            seg_len = np.diff(np.append(seg_start, ne))
            ar = np.arange(ne)
            rank = ar - np.repeat(ar[seg_start], seg_len)
        pos = qw_off[e_q, e_w] + rank

        rel16 = np.zeros(TOT, dtype=np.int16)
        slot_arr = np.full(TOT, 200.0, dtype=np.float32)
        rel16[pos] = e_rel
        slot_arr[pos] = e_slot.astype(np.float32)

        idx_wr = rel16.reshape(TOT // 16, 16).T          # [16, TOT/16]
        idx_rep = np.ascontiguousarray(np.tile(idx_wr, (8, 1)))  # [128, TOT/16]
        dstl = np.ascontiguousarray(
            slot_arr.reshape(TOTC, P).T.astype(BF16))    # [128, TOTC]

        xs = x[c * SH:(c + 1) * SH]                      # [SH, IN_DIM]
        x_t = np.zeros((IN_DIM, NSH), dtype=np.float32)
        x_t[:, :SH] = xs.T

        dv = np.zeros(NSH, dtype=np.float32)
        dv[:SH] = dinv[c * SH:(c + 1) * SH]
        dinv_t = np.ascontiguousarray(dv.reshape(NW, P).T)  # [128, NW]

        in_maps.append(dict(
            x_t=x_t, dinv_t=dinv_t, idx_rep=idx_rep, dstl=dstl,
            w1=W1f, w2=W2f, w3=W3f, b1r=b1r, b2r=b2r, b3r=b3r,
            iota=iota_bf, ident=ident,
        ))
    return meta, in_maps


def build_program(meta, repeats=1):
    IN_DIM, HID, OUT = meta["IN_DIM"], meta["HID"], meta["OUT"]
    NSH, NW, QR = meta["NSH"], meta["NW"], meta["QR"]
    SWG, granules, TOT, TOTC = meta["SWG"], meta["granules"], meta["TOT"], meta["TOTC"]
    KT = IN_DIM // P  # K tiles for transform 1
    AF = mybir.ActivationFunctionType

    nc = bacc.Bacc("TRN2", target_bir_lowering=False, debug=False, num_devices=NCORES)
    f32, bf16, i16 = mybir.dt.float32, mybir.dt.bfloat16, mybir.dt.int16

    x_t = nc.dram_tensor("x_t", [IN_DIM, NSH], f32, kind="ExternalInput")
    dinv_in = nc.dram_tensor("dinv_t", [P, NW], f32, kind="ExternalInput")
    idx_in = nc.dram_tensor("idx_rep", [P, TOT // 16], i16, kind="ExternalInput")
    dstl_in = nc.dram_tensor("dstl", [P, TOTC], bf16, kind="ExternalInput")
    w1_in = nc.dram_tensor("w1", [IN_DIM, HID], f32, kind="ExternalInput")
    w2_in = nc.dram_tensor("w2", [HID, HID], f32, kind="ExternalInput")
    w3_in = nc.dram_tensor("w3", [HID, OUT], f32, kind="ExternalInput")
    b1_in = nc.dram_tensor("b1r", [P, HID], f32, kind="ExternalInput")
    b2_in = nc.dram_tensor("b2r", [P, HID], f32, kind="ExternalInput")
    b3_in = nc.dram_tensor("b3r", [P, OUT], f32, kind="ExternalInput")
    iota_in = nc.dram_tensor("iota", [P, P], bf16, kind="ExternalInput")
    id_in = nc.dram_tensor("ident", [P, P], f32, kind="ExternalInput")
    out_ext = nc.dram_tensor("out", [NSH, OUT], f32, kind="ExternalOutput")

    with tile.TileContext(nc) as tc:
        with (
            tc.tile_pool(name="const", bufs=1) as cpool,
            tc.tile_pool(name="xload", bufs=3) as xpool,
            tc.tile_pool(name="tt", bufs=3) as ttpool,
            tc.tile_pool(name="idx", bufs=3) as ixpool,
            tc.tile_pool(name="gat", bufs=3) as gpool,
            tc.tile_pool(name="sel", bufs=2) as spool,
            tc.tile_pool(name="ev", bufs=2) as evpool,
            tc.tile_pool(name="hload", bufs=3) as hpool,
            tc.tile_pool(name="pagg", bufs=6, space="PSUM") as pagg,
            tc.tile_pool(name="pscr", bufs=2, space="PSUM") as pscr,
            tc.tile_pool(name="dram", bufs=1, space="DRAM") as dpool,
        ):
            # ---- constants ----
            iota_t = cpool.tile([P, P], bf16)
            nc.sync.dma_start(out=iota_t[:], in_=iota_in.ap())
            id_t = cpool.tile([P, P], f32)
            nc.sync.dma_start(out=id_t[:], in_=id_in.ap())
            dinv_t = cpool.tile([P, NW], f32)
            nc.sync.dma_start(out=dinv_t[:], in_=dinv_in.ap())
            w1_t = cpool.tile([P, KT, HID], f32)
            nc.sync.dma_start(
                out=w1_t[:],
                in_=w1_in.ap().rearrange("(k p) h -> p k h", p=P))
            w2_t = cpool.tile([HID, HID], f32)
            nc.sync.dma_start(out=w2_t[:], in_=w2_in.ap())
            w3_t = cpool.tile([HID, OUT], f32)
            nc.sync.dma_start(out=w3_t[:], in_=w3_in.ap())
            b1_t = cpool.tile([P, HID], f32)
            nc.sync.dma_start(out=b1_t[:], in_=b1_in.ap())
            b2_t = cpool.tile([P, HID], f32)
            nc.sync.dma_start(out=b2_t[:], in_=b2_in.ap())
            b3_t = cpool.tile([P, OUT], f32)
            nc.sync.dma_start(out=b3_t[:], in_=b3_in.ap())

            # ---- DRAM workspace ----
            tsh1 = dpool.tile([NSH, P], bf16)
            tsh2 = dpool.tile([NSH, P], bf16)
            tbl1 = dpool.tile([NCORES * NSH, P], bf16, addr_space="Shared")
            tbl2 = dpool.tile([NCORES * NSH, P], bf16, addr_space="Shared")
            h1 = dpool.tile([NSH, HID], f32)
            h2 = dpool.tile([NSH, HID], f32)

            # zero the pad columns [HID:P] of both shard tables once so the
            # collective and gathers never move uninitialized data
            zt = cpool.tile([P, NW * (P - HID)], bf16)
            nc.gpsimd.memset(zt[:], 0)
            for tsh in (tsh1, tsh2):
                nc.sync.dma_start(
                    out=tsh[:, HID:].rearrange("(w p) d -> p w d", p=P),
                    in_=zt[:].rearrange("p (w d) -> p w d", d=P - HID))

            # regroup granules by window-group start
            by_sw = {}
            for g in granules:
                by_sw.setdefault(g["w0"], []).append(g)
            # per window: total chunk count (for start/stop flags)
            wtot = {}
            for (gw0, gnw) in SWG:
                for wi in range(gnw):
                    wtot[gw0 + wi] = sum(g["wchunks"][wi] for g in by_sw[gw0])

            def transform1():
                BN = 512
                for b0 in range(0, NSH, BN):
                    bn = min(BN, NSH - b0)
                    tpsum = pscr.tile([HID, BN], f32, tag="scr")
                    for k in range(KT):
                        xk = xpool.tile([P, BN], f32, tag="xk")
                        nc.sync.dma_start(
                            out=xk[:, :bn],
                            in_=x_t.ap()[k * P:(k + 1) * P, b0:b0 + bn])
                        nc.tensor.matmul(
                            tpsum[:, :bn], w1_t[:, k, :], xk[:, :bn],
                            start=(k == 0), stop=(k == KT - 1))
                    ts = ttpool.tile([HID, BN], f32, tag="ts")
                    nc.scalar.activation(ts[:, :bn], tpsum[:, :bn], AF.Copy)
                    for j in range(bn // P):
                        wdx = (b0 + j * P) // P
                        tp2 = pscr.tile([P, HID], f32, tag="scr")
                        nc.tensor.transpose(
                            tp2[:], ts[:, j * P:(j + 1) * P], id_t[:HID, :HID])
                        tb = ttpool.tile([P, HID], bf16, tag="tb")
                        nc.scalar.activation(tb[:], tp2[:], AF.Copy,
                                             scale=dinv_t[:, wdx:wdx + 1])
                        nc.sync.dma_start(
                            out=tsh1[wdx * P:(wdx + 1) * P, :HID], in_=tb[:])

            def transform2(h_src, tsh_dst):
                for wdx in range(NW):
                    ht = hpool.tile([P, HID], f32, tag="ht")
                    nc.sync.dma_start(out=ht[:], in_=h_src[wdx * P:(wdx + 1) * P, :])
                    hT_p = pscr.tile([HID, P], f32, tag="scr")
                    nc.tensor.transpose(hT_p[:], ht[:], id_t[:])
                    hT = ttpool.tile([HID, P], f32, tag="hT")
                    nc.scalar.activation(hT[:], hT_p[:], AF.Copy)
                    t2T_p = pscr.tile([HID, P], f32, tag="scr")
                    nc.tensor.matmul(t2T_p[:], w2_t[:], hT[:], start=True, stop=True)
                    t2T = ttpool.tile([HID, P], f32, tag="t2T")
                    nc.scalar.activation(t2T[:], t2T_p[:], AF.Copy)
                    tp2 = pscr.tile([P, HID], f32, tag="scr")
                    nc.tensor.transpose(tp2[:], t2T[:], id_t[:HID, :HID])
                    tb = ttpool.tile([P, HID], bf16, tag="tbb")
                    nc.scalar.activation(tb[:], tp2[:], AF.Copy,
                                         scale=dinv_t[:, wdx:wdx + 1])
                    nc.sync.dma_start(
                        out=tsh_dst[wdx * P:(wdx + 1) * P, :HID], in_=tb[:])

            def aggregate(tbl, h_dst, bias_t):
                for (gw0, gnw) in SWG:
                    glist = by_sw[gw0]
                    # one PSUM bank per window in this group
                    pss = [pagg.tile([P, HID], f32, tag="ps", name=f"ps{gw0}_{wi}")
                           for wi in range(gnw)]
                    wseen = [0] * gnw
                    for g in glist:
                        n_idx = g["n_idx"]
                        nch = n_idx // P
                        if nch == 0:
                            continue
                        ix = ixpool.tile([P, n_idx // 16], i16, tag="ix")
                        nc.sync.dma_start(
                            out=ix[:],
                            in_=idx_in.ap()[:, g["off"] // 16:(g["off"] + n_idx) // 16])
                        gt = gpool.tile([P, nch * P], bf16, tag="gt")
                        g3 = gt[:].rearrange("p (c d) -> p c d", d=P)
                        nc.gpsimd.dma_gather(
                            out_ap=g3,
                            in_ap=tbl[g["q"] * QR:(g["q"] + 1) * QR, :],
                            idxs_ap=ix[:],
                            num_idxs=n_idx,
                            num_idxs_reg=n_idx,
                            elem_size=P,
                            elem_step=P,
                            single_packet=False,
                        )
                        dt = ixpool.tile([P, nch], bf16, tag="dt")
                        choff = g["off"] // P
                        nc.sync.dma_start(
                            out=dt[:],
                            in_=dstl_in.ap()[:, choff:choff + nch])
                        st = spool.tile([P, nch * P], bf16, tag="st")
                        s3 = st[:].rearrange("p (c q) -> p c q", q=P)
                        nc.vector.tensor_tensor(
                            out=s3,
                            in0=dt[:][:, :, None].to_broadcast([P, nch, P]),
                            in1=iota_t[:][:, None, :].to_broadcast([P, nch, P]),
                            op=mybir.AluOpType.is_equal)
                        ck = 0
                        for wi in range(g["nw"]):
                            ncw = g["wchunks"][wi]
                            tot = wtot[gw0 + wi]
                            for k in range(ncw):
                                nc.tensor.matmul(
                                    pss[wi][:],
                                    s3[:, ck, :],
                                    g3[:, ck, :HID],
                                    start=(wseen[wi] == 0),
                                    stop=(wseen[wi] == tot - 1),
                                )
                                wseen[wi] += 1
                                ck += 1
                    # evacuate: relu(dinv * psum + bias) -> h_dst rows
                    for wi in range(gnw):
                        wdx = gw0 + wi
                        ev = evpool.tile([P, HID], f32, tag="ev")
                        nc.vector.tensor_tensor(
                            out=ev[:], in0=pss[wi][:],
                            in1=dinv_t[:, wdx:wdx + 1].to_broadcast([P, HID]),
                            op=mybir.AluOpType.mult)
                        ev2 = evpool.tile([P, HID], f32, tag="ev2")
                        nc.vector.tensor_tensor(
                            out=ev2[:], in0=ev[:], in1=bias_t[:],
                            op=mybir.AluOpType.add)
                        ev4 = evpool.tile([P, HID], f32, tag="ev4")
                        nc.scalar.activation(ev4[:], ev2[:], AF.Relu)
                        nc.sync.dma_start(
                            out=h_dst[wdx * P:(wdx + 1) * P, :], in_=ev4[:])

            def output_head():
                for wdx in range(NW):
                    ht = hpool.tile([P, HID], f32, tag="ho")
                    nc.sync.dma_start(out=ht[:], in_=h2[wdx * P:(wdx + 1) * P, :])
                    hT_p = pscr.tile([HID, P], f32, tag="scr")
                    nc.tensor.transpose(hT_p[:], ht[:], id_t[:])
                    hT = ttpool.tile([HID, P], f32, tag="hTo")
                    nc.scalar.activation(hT[:], hT_p[:], AF.Copy)
                    lgT_p = pscr.tile([OUT, P], f32, tag="scr")
                    nc.tensor.matmul(lgT_p[:], w3_t[:], hT[:], start=True, stop=True)
                    lgT = ttpool.tile([OUT, P], f32, tag="lgT")
                    nc.scalar.activation(lgT[:], lgT_p[:], AF.Copy)
                    lg_p = pscr.tile([P, OUT], f32, tag="scr")
                    nc.tensor.transpose(lg_p[:], lgT[:], id_t[:OUT, :OUT])
                    lg = evpool.tile([P, OUT], f32, tag="lg")
                    nc.vector.tensor_tensor(out=lg[:], in0=lg_p[:], in1=b3_t[:],
                                            op=mybir.AluOpType.add)
                    nmax = evpool.tile([P, 1], f32, tag="nmax")
                    nc.vector.reduce_max(nmax[:], lg[:], axis=mybir.AxisListType.X,
                                         negate=True)
                    ex = evpool.tile([P, OUT], f32, tag="ex")
                    nc.scalar.activation(ex[:], lg[:], AF.Exp, bias=nmax[:, :1])
                    ssum = evpool.tile([P, 1], f32, tag="ssum")
                    nc.vector.reduce_sum(ssum[:], ex[:], axis=mybir.AxisListType.X)
                    lns = evpool.tile([P, 1], f32, tag="lns")
                    nc.scalar.activation(lns[:], ssum[:], AF.Ln)
                    tA = evpool.tile([P, OUT], f32, tag="tA")
                    nc.vector.tensor_tensor(
                        out=tA[:], in0=lg[:],
                        in1=nmax[:][:, :1].to_broadcast([P, OUT]),
                        op=mybir.AluOpType.add)
                    tB = evpool.tile([P, OUT], f32, tag="tB")
                    nc.vector.tensor_tensor(
                        out=tB[:], in0=tA[:],
                        in1=lns[:][:, :1].to_broadcast([P, OUT]),
                        op=mybir.AluOpType.subtract)
                    nc.sync.dma_start(out=out_ext.ap()[wdx * P:(wdx + 1) * P, :],
                                      in_=tB[:])

            for _rep in range(repeats):
                transform1()
                nc.gpsimd.collective_compute(
                    "AllGather", mybir.AluOpType.bypass,
                    replica_groups=[list(range(NCORES))],
                    ins=[tsh1.opt()], outs=[tbl1.opt()])
                aggregate(tbl1, h1, b1_t)
                transform2(h1, tsh2)
                nc.gpsimd.collective_compute(
                    "AllGather", mybir.AluOpType.bypass,
                    replica_groups=[list(range(NCORES))],
                    ins=[tsh2.opt()], outs=[tbl2.opt()])
                aggregate(tbl2, h2, b2_t)
                output_head()

    nc.compile()
    return nc


def postprocess(meta, results):
    SH = meta["SH"]
    outs = [np.asarray(r["out"])[:SH] for r in results]
    return np.concatenate(outs, axis=0)


from concourse.bass_utils import run_bass_kernel_spmd


def kernel(x, edge_index, W1, b1, W2, b2, W3, b3):
    x = np.asarray(x)
    edge_index = np.asarray(edge_index)
    meta, in_maps = preprocess(x, edge_index, W1, b1, W2, b2, W3, b3)
    nc = build_program(meta)
    res = run_bass_kernel_spmd(nc, in_maps, list(range(NCORES)))
    out = postprocess(meta, res.results)
    return out.astype(np.float32)
